# revision 1
# baseline (speedup 1.0000x reference)
"""Trainium2 Bass kernel for nn_EnhancedAttention (sparse axial attention +
SE + local-conv gating, fused output scale).

Sharding: pure data-parallel over batch B=32 across 8 cores (4 images/core);
tiny weights replicated. Inside each core, per image:

  - global SE gate:  sum(x) (gpsimd) -> tiny MLP (PE) -> tanh gate
  - local conv gate: dw(1x3) + dw(3x1) as diagonal-lhsT matmuls on PE with
    shifted rhs APs accumulating in PSUM, exact GELU between stages (ACT,
    bias folded into activation), pw1 (PE) -> GELU+accum -> pw2 -> tanh gate
  - axial attention: q/k projections quad-replicated across partition blocks
    via padded lhsT (enables 4-way tile_position concurrency of the K=16
    S^T matmuls), exp softmax without max-subtraction (scores bounded ~5),
    denominators via ones-rhs matmuls sharing the expS lhsT, v projected
    per-row directly in [w, d] layout (v bias folded into col/ax biases on
    host), attn@v pairs packed even/odd across partition halves,
    PE transposes + scatter copies to rebuild the [d, pixels] layouts,
    tanh (for sigmoid) with fused accumulated mean on ACT
  - fusion: all sigmoid gates computed as 0.5 + 0.5*tanh(z/2) with the
    affine part folded into host-precomputed fusion constants

All fp32. ACT table sets used: gelu_and_others + exp_and_others (tanh/relu
present in both, so 2 table loads per image).
"""

import numpy as np

B, C, H, W = 32, 256, 64, 64
MID = 16
NCORES = 8
IMGS = B // NCORES  # 4
HW = H * W  # 4096
CT = 2  # channel tiles of 128

_cache = {}


# ----------------------------------------------------------------------------
# Host-side weight preparation
# ----------------------------------------------------------------------------
def host_prep(inp):
    f32 = np.float32
    p = {}
    row_w = np.asarray(inp["row_w"], f32)   # [48, 256]
    row_b = np.asarray(inp["row_b"], f32)
    col_w = np.asarray(inp["col_w"], f32)   # [48, 16]
    col_b = np.asarray(inp["col_b"], f32)
    ax_w = np.asarray(inp["ax_w"], f32)     # [256, 16]
    ax_b = np.asarray(inp["ax_b"], f32)

    # qkv_wT[ct]: [128, 48] = (q | k | v) transposed weight slices
    qkv_wT = row_w.T.reshape(CT, 128, 48).copy()
    # padded-replica projection weights: [128c, 112] with q at cols 32r:32r+16
    qrep = np.zeros((C, 128), f32)
    krep = np.zeros((C, 128), f32)
    for r in range(4):
        qrep[:, 32 * r:32 * r + 16] = row_w[0:16].T
        krep[:, 32 * r:32 * r + 16] = row_w[16:32].T
    p["qrep_wT"] = qrep.reshape(CT, 128, 128).copy()
    p["krep_wT"] = krep.reshape(CT, 128, 128).copy()
    p["qkv_wT"] = qkv_wT
    qkb = np.zeros((128, 2), f32)
    for r in range(4):
        qkb[32 * r:32 * r + 16, 0] = row_b[0:16]
        qkb[32 * r:32 * r + 16, 1] = row_b[16:32]
    p["qkb_rep"] = qkb
    row_vb = row_b[32:48]

    # col stage (v bias folded)
    w2 = np.zeros((16, 48), f32)
    w2[:, 0:16] = col_w[0:16].T
    w2[:, 16:32] = col_w[16:32].T
    w2[:, 32:48] = col_w[32:48].T
    p["w2"] = w2
    q2w32 = np.zeros((16, 32), f32)
    q2w32[:, 0:16] = col_w[0:16].T
    k2w32 = np.zeros((16, 32), f32)
    k2w32[:, 0:16] = col_w[16:32].T
    p["q2w32"] = q2w32
    p["k2w32"] = k2w32
    q2k2b = np.zeros((128, 2), f32)
    for r in range(4):
        q2k2b[32 * r:32 * r + 16, 0] = col_b[0:16] + col_w[0:16] @ row_vb
        q2k2b[32 * r:32 * r + 16, 1] = col_b[16:32] + col_w[16:32] @ row_vb
    p["q2k2b_rep"] = q2k2b
    col_vb = col_b[32:48] + col_w[32:48] @ row_vb

    p["ax_wT"] = ax_w.T.copy()  # [16, 256]
    axb = ax_b + ax_w @ col_vb
    p["axb_half"] = (0.5 * axb).reshape(CT, 128, 1).copy()

    # conv branch
    dw1 = np.asarray(inp["dw1_w"], f32)[:, 0, 0, :]  # [256, 3]
    dw2 = np.asarray(inp["dw2_w"], f32)[:, 0, :, 0]  # [256, 3]
    dwd = np.zeros((2, 3, CT, 128, 128), f32)
    for ct in range(CT):
        for tap in range(3):
            dwd[0, tap, ct] = np.diag(dw1[128 * ct:128 * (ct + 1), tap])
            dwd[1, tap, ct] = np.diag(dw2[128 * ct:128 * (ct + 1), tap])
    p["dwdiag"] = dwd
    # negated dw1 left/right taps for w-boundary corrections (flat-shift fixup)
    dwn = np.zeros((2, CT, 128, 1), f32)
    for ct in range(CT):
        dwn[0, ct, :, 0] = -dw1[128 * ct:128 * (ct + 1), 0]
        dwn[1, ct, :, 0] = -dw1[128 * ct:128 * (ct + 1), 2]
    p["dwd1neg"] = dwn
    p["dwb"] = np.stack([
        np.asarray(inp["dw1_b"], f32).reshape(CT, 128, 1),
        np.asarray(inp["dw2_b"], f32).reshape(CT, 128, 1),
    ])  # [2, CT, 128, 1]
    p["pw1_wT"] = np.asarray(inp["pw1_w"], f32)[:, :, 0, 0].T.reshape(CT, 128, 16).copy()
    p["pw1b"] = np.asarray(inp["pw1_b"], f32).reshape(16, 1)
    p["pw2_wT"] = (np.asarray(inp["pw2_w"], f32)[:, :, 0, 0] / HW).T.copy()  # [16, 256]
    p["pw2b_half"] = (0.5 * np.asarray(inp["pw2_b"], f32)).reshape(CT, 128, 1).copy()

    # SE
    p["fc1_wT"] = (np.asarray(inp["fc1_w"], f32) / HW).T.reshape(CT, 128, 16).copy()
    p["fc1b"] = np.asarray(inp["fc1_b"], f32).reshape(16, 1)
    p["fc2_wT"] = np.asarray(inp["fc2_w"], f32).T.copy()  # [16, 256]
    p["fc2b_half"] = (0.5 * np.asarray(inp["fc2_b"], f32)).reshape(CT, 128, 1).copy()

    p["ident"] = np.eye(128, dtype=f32)

    fwin = np.asarray(inp["fusion_w"], np.float64)
    e = np.exp(fwin - fwin.max())
    fw = e / e.sum()
    p["_K0"] = float(0.5 * (fw[0] + fw[1] + fw[2]) + fw[3])
    p["_s_g"] = float(0.5 * fw[0])
    p["_s_l"] = float(0.5 * fw[1])
    p["_s_ax"] = float(0.5 * fw[2] / HW)
    return p


# ----------------------------------------------------------------------------
# Bass kernel construction
# ----------------------------------------------------------------------------
def build_nc(scalars, n_imgs=IMGS, sum_engine="vector", do_se=True,
             do_conv=True, do_att=2):
    import concourse.bacc as bacc
    import concourse.bass as bass
    import concourse.tile as tile
    from concourse import mybir

    f32 = mybir.dt.float32
    AX = mybir.AxisListType.X
    OP = mybir.AluOpType
    AF = mybir.ActivationFunctionType

    nc = bacc.Bacc("TRN2", target_bir_lowering=False, debug=False,
                   num_devices=NCORES)

    # ---- DRAM tensors ----
    dx = nc.dram_tensor("x", [n_imgs, C, HW], f32, kind="ExternalInput")
    dout = nc.dram_tensor("out", [n_imgs, C, HW], f32, kind="ExternalOutput")
    dw_names = [
        ("dwdiag", [2, 3, CT, 128, 128]), ("dwb", [2, CT, 128, 1]),
        ("dwd1neg", [2, CT, 128, 1]),
        ("qrep_wT", [CT, 128, 128]), ("krep_wT", [CT, 128, 128]),
        ("qkv_wT", [CT, 128, 48]), ("qkb_rep", [128, 2]),
        ("w2", [16, 48]), ("q2w32", [16, 32]), ("k2w32", [16, 32]),
        ("q2k2b_rep", [128, 2]),
        ("ax_wT", [16, 256]), ("axb_half", [CT, 128, 1]),
        ("pw1_wT", [CT, 128, 16]), ("pw1b", [16, 1]),
        ("pw2_wT", [16, 256]), ("pw2b_half", [CT, 128, 1]),
        ("fc1_wT", [CT, 128, 16]), ("fc1b", [16, 1]),
        ("fc2_wT", [16, 256]), ("fc2b_half", [CT, 128, 1]),
        ("ident", [128, 128]),
    ]
    dws = {nm: nc.dram_tensor(nm, sh, f32, kind="ExternalInput")
           for nm, sh in dw_names}

    K0, s_g, s_l, s_ax = (scalars["_K0"], scalars["_s_g"],
                          scalars["_s_l"], scalars["_s_ax"])

    from contextlib import ExitStack
    with tile.TileContext(nc) as tc, ExitStack() as es:
        singles = es.enter_context(tc.tile_pool(name="singles", bufs=1))
        xp = es.enter_context(tc.tile_pool(name="xp", bufs=2))
        y1p = es.enter_context(tc.tile_pool(name="y1p", bufs=1))
        xcp = es.enter_context(tc.tile_pool(name="xcp", bufs=1))
        qkp = es.enter_context(tc.tile_pool(name="qkp", bufs=1))
        scr = es.enter_context(tc.tile_pool(name="scr", bufs=2))
        att = es.enter_context(tc.tile_pool(name="att", bufs=1))
        attS = es.enter_context(tc.tile_pool(name="attS", bufs=8))
        tiny = es.enter_context(tc.tile_pool(name="tiny", bufs=4))
        ps_mm = es.enter_context(tc.tile_pool(name="ps_mm", bufs=2, space="PSUM"))
        ps_S = es.enter_context(tc.tile_pool(name="ps_S", bufs=2, space="PSUM"))
        ps_O = es.enter_context(tc.tile_pool(name="ps_O", bufs=2, space="PSUM"))
        ps_sm = es.enter_context(tc.tile_pool(name="ps_sm", bufs=2, space="PSUM"))

        # ---- load weights to SBUF ----
        def wtile(name, shape, src):
            t = singles.tile(shape, f32, tag=name)
            nc.sync.dma_start(out=t[:], in_=src)
            return t

        dwd_sb = [[[wtile(f"dwd{st}{tap}{ct}", [128, 128],
                          dws["dwdiag"][st, tap, ct])
                    for ct in range(CT)] for tap in range(3)] for st in range(2)]
        dwb_sb = [[wtile(f"dwb{st}{ct}", [128, 1], dws["dwb"][st, ct])
                   for ct in range(CT)] for st in range(2)]
        dwn_sb = [[wtile(f"dwn{sd}{ct}", [128, 1], dws["dwd1neg"][sd, ct])
                   for ct in range(CT)] for sd in range(2)]
        qrep_sb = [wtile(f"qrep{ct}", [128, 128], dws["qrep_wT"][ct]) for ct in range(CT)]
        krep_sb = [wtile(f"krep{ct}", [128, 128], dws["krep_wT"][ct]) for ct in range(CT)]
        qkv_sb = [wtile(f"qkv{ct}", [128, 48], dws["qkv_wT"][ct]) for ct in range(CT)]
        qkb_sb = wtile("qkb", [128, 2], dws["qkb_rep"][:])
        w2_sb = wtile("w2", [16, 48], dws["w2"][:])
        q2w32_sb = wtile("q2w32", [16, 32], dws["q2w32"][:])
        k2w32_sb = wtile("k2w32", [16, 32], dws["k2w32"][:])
        q2k2b_sb = wtile("q2k2b", [128, 2], dws["q2k2b_rep"][:])
        ax_wT_sb = wtile("axwT", [16, 256], dws["ax_wT"][:])
        axbh_sb = [wtile(f"axbh{ct}", [128, 1], dws["axb_half"][ct]) for ct in range(CT)]
        pw1_sb = [wtile(f"pw1{ct}", [128, 16], dws["pw1_wT"][ct]) for ct in range(CT)]
        pw1b_sb = wtile("pw1b", [16, 1], dws["pw1b"][:])
        pw2_sb = wtile("pw2", [16, 256], dws["pw2_wT"][:])
        pw2bh_sb = [wtile(f"pw2bh{ct}", [128, 1], dws["pw2b_half"][ct]) for ct in range(CT)]
        fc1_sb = [wtile(f"fc1{ct}", [128, 16], dws["fc1_wT"][ct]) for ct in range(CT)]
        fc1b_sb = wtile("fc1b", [16, 1], dws["fc1b"][:])
        fc2_sb = wtile("fc2", [16, 256], dws["fc2_wT"][:])
        fc2bh_sb = [wtile(f"fc2bh{ct}", [128, 1], dws["fc2b_half"][ct]) for ct in range(CT)]
        ident_sb = wtile("ident", [128, 128], dws["ident"][:])
        ones_sb = singles.tile([128, 1], f32, tag="ones", name="ones")
        nc.vector.memset(ones_sb[:], 1.0)

        for i in range(n_imgs):
            # ================= load x =================
            x = [xp.tile([128, HW], f32, tag=f"x{ct}", name=f"x{ct}") for ct in range(CT)]
            for ct in range(CT):
                nc.sync.dma_start(out=x[ct][:], in_=dx[i, 128 * ct:128 * (ct + 1), :])

            # ================= global SE gate =================
            tg = []
            if do_se:
                gsum = [tiny.tile([128, 1], f32, tag="gsum", name="gsum") for _ in range(CT)]
                for ct in range(CT):
                    eng = nc.gpsimd if sum_engine == "gpsimd" else nc.vector
                    eng.reduce_sum(out=gsum[ct][:], in_=x[ct][:], axis=AX)
                fc1ps = ps_sm.tile([16, 1], f32, tag="sm", name="sm")
                for ct in range(CT):
                    nc.tensor.matmul(fc1ps[:], fc1_sb[ct][:], gsum[ct][:],
                                     start=(ct == 0), stop=(ct == 1))
                r1 = tiny.tile([16, 1], f32, tag="r1", name="r1")
                nc.scalar.activation(out=r1[:], in_=fc1ps[:], func=AF.Relu,
                                     bias=fc1b_sb[:], scale=1.0)
                for ct in range(CT):
                    fc2ps = ps_sm.tile([128, 1], f32, tag="sm", name="sm")
                    nc.tensor.matmul(fc2ps[:], fc2_sb[:, 128 * ct:128 * (ct + 1)], r1[:])
                    t = tiny.tile([128, 1], f32, tag="tg", name="tg")
                    nc.scalar.activation(out=t[:], in_=fc2ps[:], func=AF.Tanh,
                                         bias=fc2bh_sb[ct][:], scale=0.5)
                    tg.append(t)
            else:
                for ct in range(CT):
                    t = tiny.tile([128, 1], f32, tag="tg", name="tg")
                    nc.vector.memset(t[:], 0.0)
                    tg.append(t)

            if do_conv:
                # ================= conv branch: dw1 -> gelu -> y1 =================
                y1 = [y1p.tile([128, HW], f32, tag=f"y1{ct}", name=f"y1{ct}") for ct in range(CT)]
                for ct in range(CT):
                    x3 = x[ct].rearrange("p (h w) -> p h w", w=64)
                    for c in range(8):
                        o = 512 * c
                        ps = ps_mm.tile([128, 512], f32, tag="mm", name="mm")
                        ps3 = ps.rearrange("p (h w) -> p h w", w=64)
                        # flat shifted taps; h-row wrap fixed by corrections below
                        nc.tensor.matmul(ps[:], dwd_sb[0][1][ct][:], x[ct][:, o:o + 512],
                                         start=True, stop=False)
                        lo = 1 if c == 0 else 0
                        nc.tensor.matmul(ps[:, lo:512], dwd_sb[0][0][ct][:],
                                         x[ct][:, o + lo - 1:o + 511],
                                         start=False, stop=False)
                        hi = 511 if c == 7 else 512
                        nc.tensor.matmul(ps[:, 0:hi], dwd_sb[0][2][ct][:],
                                         x[ct][:, o + 1:o + 1 + hi],
                                         start=False, stop=True)
                        # subtract wrapped left tap at w=0 (h>0), right tap at w=63
                        lh = 1 if c == 0 else 0
                        nc.vector.scalar_tensor_tensor(
                            out=ps3[:, lh:8, 0], in0=x3[:, 8 * c + lh - 1:8 * c + 7, 63],
                            scalar=dwn_sb[0][ct][:], in1=ps3[:, lh:8, 0],
                            op0=OP.mult, op1=OP.add)
                        rh = 7 if c == 7 else 8
                        nc.vector.scalar_tensor_tensor(
                            out=ps3[:, 0:rh, 63], in0=x3[:, 8 * c + 1:8 * c + 1 + rh, 0],
                            scalar=dwn_sb[1][ct][:], in1=ps3[:, 0:rh, 63],
                            op0=OP.mult, op1=OP.add)
                        nc.scalar.activation(out=y1[ct][:, o:o + 512], in_=ps[:],
                                             func=AF.Gelu, bias=dwb_sb[0][ct][:], scale=1.0)

                # ========== dw2 -> gelu -> y2 chunks -> pw1 -> gelu+accum ==========
                lsum_cols = tiny.tile([16, 8], f32, tag="lsum_cols", name="lsum_cols")
                for c in range(8):
                    o = 512 * c
                    y2c = []
                    for ct in range(CT):
                        ps = ps_mm.tile([128, 512], f32, tag="mm", name="mm")
                        nc.tensor.matmul(ps[:], dwd_sb[1][1][ct][:], y1[ct][:, o:o + 512],
                                         start=True, stop=False)
                        if c == 0:
                            nc.tensor.matmul(ps[:, 64:512], dwd_sb[1][0][ct][:],
                                             y1[ct][:, 0:448], start=False, stop=False)
                        else:
                            nc.tensor.matmul(ps[:], dwd_sb[1][0][ct][:],
                                             y1[ct][:, o - 64:o + 448],
                                             start=False, stop=False)
                        if c == 7:
                            nc.tensor.matmul(ps[:, 0:448], dwd_sb[1][2][ct][:],
                                             y1[ct][:, o + 64:o + 512],
                                             start=False, stop=True)
                        else:
                            nc.tensor.matmul(ps[:], dwd_sb[1][2][ct][:],
                                             y1[ct][:, o + 64:o + 576],
                                             start=False, stop=True)
                        yc = scr.tile([128, 512], f32, tag=f"y2c{ct}", name=f"y2c{ct}")
                        nc.scalar.activation(out=yc[:], in_=ps[:], func=AF.Gelu,
                                             bias=dwb_sb[1][ct][:], scale=1.0)
                        y2c.append(yc)
                    pps = ps_mm.tile([16, 512], f32, tag="mm", name="mm")
                    for ct in range(CT):
                        nc.tensor.matmul(pps[:], pw1_sb[ct][:], y2c[ct][:],
                                         start=(ct == 0), stop=(ct == 1))
                    g3 = scr.tile([16, 512], f32, tag="g3", name="g3")
                    nc.scalar.activation(out=g3[:], in_=pps[:], func=AF.Gelu,
                                         bias=pw1b_sb[:], scale=1.0,
                                         accum_out=lsum_cols[:, c:c + 1])

                # local gate
                lsum = tiny.tile([16, 1], f32, tag="lsum", name="lsum")
                nc.vector.reduce_sum(out=lsum[:], in_=lsum_cols[:], axis=AX)
                tl = []
                for ct in range(CT):
                    ps = ps_sm.tile([128, 1], f32, tag="sm", name="sm")
                    nc.tensor.matmul(ps[:], pw2_sb[:, 128 * ct:128 * (ct + 1)], lsum[:])
                    t = tiny.tile([128, 1], f32, tag="tl", name="tl")
                    nc.scalar.activation(out=t[:], in_=ps[:], func=AF.Tanh,
                                         bias=pw2bh_sb[ct][:], scale=0.5)
                    tl.append(t)
            else:
                tl = []
                for ct in range(CT):
                    t = tiny.tile([128, 1], f32, tag="tl", name="tl")
                    nc.vector.memset(t[:], 0.0)
                    tl.append(t)

            if do_att >= 1:
                # ================= row attention =================
                # q/k projections, quad-replicated partition blocks
                q_sb = qkp.tile([128, 1024], f32, tag="q", name="q")
                k_sb = qkp.tile([128, 1024], f32, tag="k", name="k")
                for g in range(2):
                    for which, rep, dst, bcol in ((0, qrep_sb, q_sb, 0),
                                                  (1, krep_sb, k_sb, 1)):
                        ps = ps_mm.tile([128, 512], f32, tag="mm", name="mm")
                        for r in range(4):
                            c = 4 * g + r
                            for ct in range(CT):
                                nc.tensor.matmul(
                                    ps[32 * r:32 * r + 32, :],
                                    rep[ct][:, 32 * r:32 * r + 32],
                                    x[ct][:, 512 * c:512 * c + 512],
                                    start=(ct == 0), stop=(ct == 1),
                                    tile_position=(0, 32 * r))
                        nc.vector.tensor_scalar(
                            out=dst[:, 512 * g:512 * g + 512], in0=ps[:],
                            scalar1=qkb_sb[:, bcol:bcol + 1], scalar2=None,
                            op0=OP.add)

                # v-direct: [w, d] layout, pairs packed even/odd partition halves
                vt_sb = att.tile([128, 544], f32, tag="vt", name="vt")
                vt3 = vt_sb.rearrange("p (j c) -> p j c", c=17)
                nc.vector.memset(vt3[:, :, 16], 1.0)
                for g4 in range(8):
                    vps = ps_sm.tile([128, 64], f32, tag="sm", name="sm")
                    for p in range(4):
                        j = 4 * g4 + p
                        for h in (2 * j, 2 * j + 1):
                            for ct in range(CT):
                                nc.tensor.matmul(
                                    vps[64 * (h % 2):64 * (h % 2) + 64, 16 * p:16 * p + 16],
                                    x[ct][:, 64 * h:64 * h + 64],
                                    qkv_sb[ct][:, 32:48],
                                    start=(ct == 0), stop=(ct == 1),
                                    tile_position=(0, 64 * (h % 2)))
                    vsrc = vps.rearrange("p (j c) -> p j c", c=16)
                    nc.vector.tensor_copy(out=vt3[:, 4 * g4:4 * g4 + 4, 0:16], in_=vsrc)

                def attention_block(qt, kt, vtt, OC_dst):
                    """S^T matmuls -> exp -> attn@v + denom -> normalize.
                    One S-tile per 64-px chunk so all matmuls in a PSUM bank
                    share one row group (HW: different row groups must not
                    write the same bank)."""
                    vt3l = vtt.rearrange("p (j c) -> p j c", c=17)
                    for t in range(4):
                        expSs = []
                        for half in range(4):
                            cch = 2 * t + half // 2
                            r, g = cch % 4, cch // 4
                            Sps = ps_S.tile([128, 256], f32, tag="S", name="S")
                            for u in range(2):
                                j = 4 * cch + 2 * (half % 2) + u
                                h0 = 2 * j
                                sl = slice(32 * r, 32 * r + 16)
                                fo = 512 * g + 64 * (h0 % 8)
                                # merged even/odd pair matmul: diag blocks are
                                # S^T(h0) rows 0:64 / S^T(h0+1) rows 64:128
                                nc.tensor.matmul(
                                    Sps[:, 128 * u:128 * u + 128],
                                    kt[sl, fo:fo + 128], qt[sl, fo:fo + 128],
                                    tile_position=(32 * r, 0))
                            expS = attS.tile([128, 256], f32, tag="expS", name="expS")
                            nc.scalar.activation(out=expS[:], in_=Sps[:], func=AF.Exp,
                                                 scale=0.25)
                            expSs.append(expS)
                        Ops = ps_O.tile([128, 136], f32, tag="O", name="O")
                        for s in range(8):
                            j = 8 * t + s
                            expS = expSs[s // 2]
                            u = j % 2
                            for dh in range(2):
                                sl = slice(64 * dh, 64 * dh + 64)
                                E = expS[sl, 128 * u + 64 * dh:128 * u + 64 * dh + 64]
                                nc.tensor.matmul(
                                    Ops[sl, 17 * s:17 * s + 17], E,
                                    vt3l[sl, j, :],
                                    tile_position=(64 * dh, 64 * dh))
                        O3 = Ops.rearrange("p (s c) -> p s c", c=17)
                        rD = tiny.tile([128, 8], f32, tag="rD", name="rD")
                        nc.vector.reciprocal(out=rD[:], in_=O3[:, :, 16])
                        import concourse.bass as bass_mod
                        rDb = bass_mod.AP(tensor=rD.tensor, offset=rD.offset,
                                          ap=[rD.ap[0], [1, 8], [0, 16]])
                        dst3 = OC_dst[:, 128 * t:128 * t + 128].rearrange(
                            "p (s c) -> p s c", c=16)
                        nc.vector.tensor_tensor(out=dst3[:], in0=O3[:, :, 0:16],
                                                in1=rDb, op=OP.mult)

                OR_sb = att.tile([128, 512], f32, tag="OR", name="OR")
                attention_block(q_sb, k_sb, vt_sb, OR_sb)

                # transposeA + scatter -> XC [16, w*64+h] (w-major)
                XC = xcp.tile([16, HW], f32, tag="XC", name="XC")
                XC3 = XC.rearrange("d (w h) -> d w h", h=64)
                for j in range(32):
                    trp = ps_sm.tile([16, 128], f32, tag="sm", name="tr")
                    nc.tensor.transpose(trp[:], OR_sb[:, 16 * j:16 * j + 16],
                                        ident_sb[:])
                    tsrc = trp.rearrange("d (t w) -> d w t", t=2)
                    nc.vector.tensor_copy(out=XC3[:, :, 2 * j:2 * j + 2], in_=tsrc)

            if do_att >= 2:
                # ================= col attention =================
                q2_sb = qkp.tile([128, 1024], f32, tag="q2", name="q2")
                k2_sb = qkp.tile([128, 1024], f32, tag="k2", name="k2")
                for g in range(2):
                    for w32, dst, bcol in ((q2w32_sb, q2_sb, 0),
                                           (k2w32_sb, k2_sb, 1)):
                        ps = ps_mm.tile([128, 512], f32, tag="mm", name="mm")
                        for r in range(4):
                            c = 4 * g + r
                            nc.tensor.matmul(ps[32 * r:32 * r + 32, :],
                                             w32[:],
                                             XC[:, 512 * c:512 * c + 512],
                                             tile_position=(0, 32 * r))
                        nc.vector.tensor_scalar(
                            out=dst[:, 512 * g:512 * g + 512], in0=ps[:],
                            scalar1=q2k2b_sb[:, bcol:bcol + 1], scalar2=None,
                            op0=OP.add)

                vt2_sb = att.tile([128, 544], f32, tag="vt2", name="vt2")
                vt23 = vt2_sb.rearrange("p (j c) -> p j c", c=17)
                nc.vector.memset(vt23[:, :, 16], 1.0)
                for g4 in range(8):
                    vps = ps_sm.tile([128, 64], f32, tag="sm", name="sm")
                    for p in range(4):
                        j = 4 * g4 + p
                        for w in (2 * j, 2 * j + 1):
                            nc.tensor.matmul(
                                vps[64 * (w % 2):64 * (w % 2) + 64, 16 * p:16 * p + 16],
                                XC[:, 64 * w:64 * w + 64], w2_sb[:, 32:48],
                                tile_position=(0, 64 * (w % 2)))
                    vsrc = vps.rearrange("p (j c) -> p j c", c=16)
                    nc.vector.tensor_copy(out=vt23[:, 4 * g4:4 * g4 + 4, 0:16], in_=vsrc)

                OC_sb = att.tile([128, 512], f32, tag="OC", name="OC")
                attention_block(q2_sb, k2_sb, vt2_sb, OC_sb)

                # transposeB + contiguous scatter -> XC2 [16, pixels]
                XC2 = xcp.tile([16, HW], f32, tag="XC2", name="XC2")
                for j in range(32):
                    trp = ps_sm.tile([16, 128], f32, tag="sm", name="tr")
                    nc.tensor.transpose(trp[:], OC_sb[:, 16 * j:16 * j + 16],
                                        ident_sb[:])
                    nc.vector.tensor_copy(out=XC2[:, 128 * j:128 * j + 128],
                                          in_=trp[:])

                # ax projection + tanh + accumulated mean
                ta_cols = [tiny.tile([128, 8], f32, tag="ta_cols", name="ta_cols") for _ in range(CT)]
                for ct in range(CT):
                    for c in range(8):
                        ps = ps_mm.tile([128, 512], f32, tag="mm", name="mm")
                        nc.tensor.matmul(ps[:], ax_wT_sb[:, 128 * ct:128 * (ct + 1)],
                                         XC2[:, 512 * c:512 * c + 512])
                        axs = scr.tile([128, 512], f32, tag="axs", name="axs")
                        nc.scalar.activation(out=axs[:], in_=ps[:], func=AF.Tanh,
                                             bias=axbh_sb[ct][:], scale=0.5,
                                             accum_out=ta_cols[ct][:, c:c + 1])

            if do_att < 2:
                ta_cols = [tiny.tile([128, 8], f32, tag="ta_cols", name="ta_cols") for _ in range(CT)]
                for ct in range(CT):
                    nc.vector.memset(ta_cols[ct][:], 0.0)
                if do_att == 1:
                    nc.sync.dma_start(out=dout[i, 0:16, :], in_=XC[:])

            # ================= fusion + final =================
            for ct in range(CT):
                ta = tiny.tile([128, 1], f32, tag="ta", name="ta")
                nc.vector.reduce_sum(out=ta[:], in_=ta_cols[ct][:], axis=AX)
                f0 = tiny.tile([128, 1], f32, tag="f0", name="f0")
                nc.vector.tensor_scalar(out=f0[:], in0=tg[ct][:], scalar1=s_g,
                                        scalar2=K0, op0=OP.mult, op1=OP.add)
                f1 = tiny.tile([128, 1], f32, tag="f1", name="f1")
                nc.vector.scalar_tensor_tensor(out=f1[:], in0=tl[ct][:],
                                               scalar=s_l, in1=f0[:],
                                               op0=OP.mult, op1=OP.add)
                fin = tiny.tile([128, 1], f32, tag="fin", name="fin")
                nc.vector.scalar_tensor_tensor(out=fin[:], in0=ta[:],
                                               scalar=s_ax, in1=f1[:],
                                               op0=OP.mult, op1=OP.add)
                nc.vector.tensor_scalar(out=x[ct][:], in0=x[ct][:],
                                        scalar1=fin[:], scalar2=None,
                                        op0=OP.mult)
                nc.sync.dma_start(out=dout[i, 128 * ct:128 * (ct + 1), :],
                                  in_=x[ct][:])

    nc.compile()
    return nc


# ----------------------------------------------------------------------------
# Entry point
# ----------------------------------------------------------------------------
def kernel(**inputs):
    from concourse.bass_utils import run_bass_kernel_spmd

    p = host_prep(inputs)
    key = "nc"
    if key not in _cache:
        _cache[key] = build_nc(p)
    nc = _cache[key]

    x = np.asarray(inputs["x"], np.float32).reshape(B, C, HW)
    wmap = {nm: p[nm] for nm in (
        "dwdiag", "dwb", "dwd1neg", "qrep_wT", "krep_wT", "qkv_wT", "qkb_rep", "w2",
        "q2w32", "k2w32", "q2k2b_rep", "ax_wT", "axb_half", "pw1_wT", "pw1b", "pw2_wT",
        "pw2b_half", "fc1_wT", "fc1b", "fc2_wT", "fc2b_half", "ident")}
    in_maps = [{"x": x[IMGS * c:IMGS * (c + 1)], **wmap} for c in range(NCORES)]
    res = run_bass_kernel_spmd(nc, in_maps, list(range(NCORES)))
    _cache["last_results"] = res
    out = np.concatenate([res.results[c]["out"] for c in range(NCORES)], axis=0)
    return out.reshape(B, C, H, W)



# revision 6
# speedup vs baseline: 1.8696x; 1.8696x over previous
"""Trainium2 Bass kernel for nn_EnhancedAttention (sparse axial attention +
SE + local-conv gating, fused output scale).

Sharding: pure data-parallel over batch B=32 across 8 cores (4 images/core);
tiny weights replicated. Inside each core, per image:

  - global SE gate:  sum(x) (DVE, bf16 2x) -> tiny MLP (PE) -> tanh gate
  - local conv gate: dw(1x3) + dw(3x1) as diagonal-lhsT matmuls on PE with
    shifted rhs APs accumulating in PSUM, exact GELU between stages (ACT,
    bias folded into activation), pw1 (PE, outputs packed 8 chunks x 16
    partitions into one [128,512] PSUM tile) -> single GELU+accum ->
    mask-matmul partition reduction -> pw2 -> tanh gate
  - axial attention: q/k projections quad-replicated across partition blocks
    via padded lhsT (enables 4-way tile_position concurrency of the K=16
    S^T matmuls), exp softmax without max-subtraction (scores bounded ~5),
    denominators via ones-rhs matmuls sharing the expS lhsT, v projected
    per-row directly in [w, d] layout (v bias folded into col/ax biases on
    host), attn@v pairs packed even/odd across partition halves,
    PE transposes + scatter copies to rebuild the [d, pixels] layouts,
    tanh (for sigmoid) with fused accumulated mean on ACT
  - fusion: all sigmoid gates computed as 0.5 + 0.5*tanh(z/2) with the
    affine part folded into host-precomputed fusion constants

Data path is bf16 (matmuls run at 1 cyc/col vs 4 for fp32); PSUM, biases,
activation accumulators and gate scalars stay fp32. Host converts x to bf16
on the way in and the bf16 output back to fp32.
"""

import numpy as np
import ml_dtypes

B, C, H, W = 32, 256, 64, 64
MID = 16
NCORES = 8
IMGS = B // NCORES  # 4
HW = H * W  # 4096
CT = 2  # channel tiles of 128

BF = ml_dtypes.bfloat16

_cache = {}

# weights fed to matmuls (bf16) vs bias/scalar APs (fp32)
BF16_W = ("dwdiag", "qrep_wT", "krep_wT", "qkv_wT", "w2", "q2w32", "k2w32",
          "ax_wT", "pw1_wT", "ident")
F32_W = ("dwb", "dwd1neg", "qkb_rep", "q2k2b_rep", "axb_half", "pw1b_rep",
         "pw2b_half", "fc1b", "fc2b_half", "mask16", "pw2_wT", "fc1_wT",
         "fc2_wT")


# ----------------------------------------------------------------------------
# Host-side weight preparation
# ----------------------------------------------------------------------------
def host_prep(inp):
    f32 = np.float32
    p = {}
    row_w = np.asarray(inp["row_w"], f32)   # [48, 256]
    row_b = np.asarray(inp["row_b"], f32)
    col_w = np.asarray(inp["col_w"], f32)   # [48, 16]
    col_b = np.asarray(inp["col_b"], f32)
    ax_w = np.asarray(inp["ax_w"], f32)     # [256, 16]
    ax_b = np.asarray(inp["ax_b"], f32)

    # qkv_wT[ct]: [128, 48] = (q | k | v) transposed weight slices
    qkv_wT = row_w.T.reshape(CT, 128, 48).copy()
    # padded-replica projection weights: [128c, 112] with q at cols 32r:32r+16
    qrep = np.zeros((C, 128), f32)
    krep = np.zeros((C, 128), f32)
    for r in range(4):
        qrep[:, 32 * r:32 * r + 16] = row_w[0:16].T
        krep[:, 32 * r:32 * r + 16] = row_w[16:32].T
    p["qrep_wT"] = qrep.reshape(CT, 128, 128).copy()
    p["krep_wT"] = krep.reshape(CT, 128, 128).copy()
    p["qkv_wT"] = qkv_wT
    qkb = np.zeros((128, 2), f32)
    for r in range(4):
        qkb[32 * r:32 * r + 16, 0] = row_b[0:16]
        qkb[32 * r:32 * r + 16, 1] = row_b[16:32]
    p["qkb_rep"] = qkb
    row_vb = row_b[32:48]

    # col stage (v bias folded)
    w2 = np.zeros((16, 48), f32)
    w2[:, 0:16] = col_w[0:16].T
    w2[:, 16:32] = col_w[16:32].T
    w2[:, 32:48] = col_w[32:48].T
    p["w2"] = w2
    q2w32 = np.zeros((16, 32), f32)
    q2w32[:, 0:16] = col_w[0:16].T
    k2w32 = np.zeros((16, 32), f32)
    k2w32[:, 0:16] = col_w[16:32].T
    p["q2w32"] = q2w32
    p["k2w32"] = k2w32
    q2k2b = np.zeros((128, 2), f32)
    for r in range(4):
        q2k2b[32 * r:32 * r + 16, 0] = col_b[0:16] + col_w[0:16] @ row_vb
        q2k2b[32 * r:32 * r + 16, 1] = col_b[16:32] + col_w[16:32] @ row_vb
    p["q2k2b_rep"] = q2k2b
    col_vb = col_b[32:48] + col_w[32:48] @ row_vb

    p["ax_wT"] = ax_w.T.copy()  # [16, 256]
    axb = ax_b + ax_w @ col_vb
    p["axb_half"] = (0.5 * axb).reshape(CT, 128, 1).copy()

    # conv branch
    dw1 = np.asarray(inp["dw1_w"], f32)[:, 0, 0, :]  # [256, 3]
    dw2 = np.asarray(inp["dw2_w"], f32)[:, 0, :, 0]  # [256, 3]
    dwd = np.zeros((2, 3, CT, 128, 128), f32)
    for ct in range(CT):
        for tap in range(3):
            dwd[0, tap, ct] = np.diag(dw1[128 * ct:128 * (ct + 1), tap])
            dwd[1, tap, ct] = np.diag(dw2[128 * ct:128 * (ct + 1), tap])
    p["dwdiag"] = dwd
    # negated dw1 left/right taps for w-boundary corrections (flat-shift fixup)
    dwn = np.zeros((2, CT, 128, 1), f32)
    for ct in range(CT):
        dwn[0, ct, :, 0] = -dw1[128 * ct:128 * (ct + 1), 0]
        dwn[1, ct, :, 0] = -dw1[128 * ct:128 * (ct + 1), 2]
    p["dwd1neg"] = dwn
    p["dwb"] = np.stack([
        np.asarray(inp["dw1_b"], f32).reshape(CT, 128, 1),
        np.asarray(inp["dw2_b"], f32).reshape(CT, 128, 1),
    ])  # [2, CT, 128, 1]
    p["pw1_wT"] = np.asarray(inp["pw1_w"], f32)[:, :, 0, 0].T.reshape(CT, 128, 16).copy()
    # pw1 outputs packed 4 chunks x 32-aligned blocks -> replicate bias
    pw1b = np.asarray(inp["pw1_b"], f32)
    p["pw1b_rep"] = np.tile(pw1b, 8).reshape(128, 1).copy()
    # partition-reduction mask: lsum[m] = sum_k acc[32k + m]
    mask16 = np.zeros((128, 16), f32)
    for k in range(4):
        for m in range(16):
            mask16[32 * k + m, m] = 1.0
    p["mask16"] = mask16
    p["pw2_wT"] = (np.asarray(inp["pw2_w"], f32)[:, :, 0, 0] / HW).T.copy()  # [16, 256]
    p["pw2b_half"] = (0.5 * np.asarray(inp["pw2_b"], f32)).reshape(CT, 128, 1).copy()

    # SE
    p["fc1_wT"] = (np.asarray(inp["fc1_w"], f32) / HW).T.reshape(CT, 128, 16).copy()
    p["fc1b"] = np.asarray(inp["fc1_b"], f32).reshape(16, 1)
    p["fc2_wT"] = np.asarray(inp["fc2_w"], f32).T.copy()  # [16, 256]
    p["fc2b_half"] = (0.5 * np.asarray(inp["fc2_b"], f32)).reshape(CT, 128, 1).copy()

    p["ident"] = np.eye(128, dtype=f32)

    fwin = np.asarray(inp["fusion_w"], np.float64)
    e = np.exp(fwin - fwin.max())
    fw = e / e.sum()
    p["_K0"] = float(0.5 * (fw[0] + fw[1] + fw[2]) + fw[3])
    p["_s_g"] = float(0.5 * fw[0])
    p["_s_l"] = float(0.5 * fw[1])
    p["_s_ax"] = float(0.5 * fw[2] / HW)

    for nm in BF16_W:
        p[nm] = np.asarray(p[nm], f32).astype(BF)
    return p


# ----------------------------------------------------------------------------
# Bass kernel construction
# ----------------------------------------------------------------------------
def build_nc(scalars, n_imgs=IMGS, do_se=True, do_conv=True, do_att=2):
    import concourse.bacc as bacc
    import concourse.bass as bass
    import concourse.tile as tile
    from concourse import mybir

    f32 = mybir.dt.float32
    bf16 = mybir.dt.bfloat16
    AX = mybir.AxisListType.X
    OP = mybir.AluOpType
    AF = mybir.ActivationFunctionType

    nc = bacc.Bacc("TRN2", target_bir_lowering=False, debug=False,
                   num_devices=NCORES)

    # ---- DRAM tensors ----
    dx = nc.dram_tensor("x", [n_imgs, C, HW], bf16, kind="ExternalInput")
    dout = nc.dram_tensor("out", [n_imgs, C, HW], bf16, kind="ExternalOutput")
    dw_names = [
        ("dwdiag", [2, 3, CT, 128, 128]), ("dwb", [2, CT, 128, 1]),
        ("dwd1neg", [2, CT, 128, 1]),
        ("qrep_wT", [CT, 128, 128]), ("krep_wT", [CT, 128, 128]),
        ("qkv_wT", [CT, 128, 48]), ("qkb_rep", [128, 2]),
        ("w2", [16, 48]), ("q2w32", [16, 32]), ("k2w32", [16, 32]),
        ("q2k2b_rep", [128, 2]),
        ("ax_wT", [16, 256]), ("axb_half", [CT, 128, 1]),
        ("pw1_wT", [CT, 128, 16]), ("pw1b_rep", [128, 1]),
        ("mask16", [128, 16]),
        ("pw2_wT", [16, 256]), ("pw2b_half", [CT, 128, 1]),
        ("fc1_wT", [CT, 128, 16]), ("fc1b", [16, 1]),
        ("fc2_wT", [16, 256]), ("fc2b_half", [CT, 128, 1]),
        ("ident", [128, 128]),
    ]
    dws = {nm: nc.dram_tensor(nm, sh, bf16 if nm in BF16_W else f32,
                              kind="ExternalInput")
           for nm, sh in dw_names}

    K0, s_g, s_l, s_ax = (scalars["_K0"], scalars["_s_g"],
                          scalars["_s_l"], scalars["_s_ax"])

    from contextlib import ExitStack
    with tile.TileContext(nc) as tc, ExitStack() as es:
        singles = es.enter_context(tc.tile_pool(name="singles", bufs=1))
        xp = es.enter_context(tc.tile_pool(name="xp", bufs=2))
        y1p = es.enter_context(tc.tile_pool(name="y1p", bufs=1))
        xcp = es.enter_context(tc.tile_pool(name="xcp", bufs=1))
        qkp = es.enter_context(tc.tile_pool(name="qkp", bufs=1))
        scr = es.enter_context(tc.tile_pool(name="scr", bufs=2))
        att = es.enter_context(tc.tile_pool(name="att", bufs=1))
        attS = es.enter_context(tc.tile_pool(name="attS", bufs=8))
        tiny = es.enter_context(tc.tile_pool(name="tiny", bufs=4))
        ps_mm = es.enter_context(tc.tile_pool(name="ps_mm", bufs=2, space="PSUM"))
        ps_S = es.enter_context(tc.tile_pool(name="ps_S", bufs=2, space="PSUM"))
        ps_O = es.enter_context(tc.tile_pool(name="ps_O", bufs=2, space="PSUM"))
        ps_pw = es.enter_context(tc.tile_pool(name="ps_pw", bufs=1, space="PSUM"))
        ps_small = es.enter_context(tc.tile_pool(name="ps_small", bufs=1, space="PSUM"))

        # one shared PSUM bank for all small matmul outputs + transposes
        psmall = ps_small.tile([128, 512], f32, tag="small", name="psmall")
        psmall_bf = psmall.bitcast(bf16)

        # ---- load weights to SBUF ----
        def wtile(name, shape, src, dt):
            t = singles.tile(shape, dt, tag=name)
            nc.sync.dma_start(out=t[:], in_=src)
            return t

        dwd_sb = [[[wtile(f"dwd{st}{tap}{ct}", [128, 128],
                          dws["dwdiag"][st, tap, ct], bf16)
                    for ct in range(CT)] for tap in range(3)] for st in range(2)]
        dwb_sb = [[wtile(f"dwb{st}{ct}", [128, 1], dws["dwb"][st, ct], f32)
                   for ct in range(CT)] for st in range(2)]
        dwn_sb = [[wtile(f"dwn{sd}{ct}", [128, 1], dws["dwd1neg"][sd, ct], f32)
                   for ct in range(CT)] for sd in range(2)]
        qrep_sb = [wtile(f"qrep{ct}", [128, 128], dws["qrep_wT"][ct], bf16) for ct in range(CT)]
        krep_sb = [wtile(f"krep{ct}", [128, 128], dws["krep_wT"][ct], bf16) for ct in range(CT)]
        qkv_sb = [wtile(f"qkv{ct}", [128, 48], dws["qkv_wT"][ct], bf16) for ct in range(CT)]
        qkb_sb = wtile("qkb", [128, 2], dws["qkb_rep"][:], f32)
        w2_sb = wtile("w2", [16, 48], dws["w2"][:], bf16)
        q2w32_sb = wtile("q2w32", [16, 32], dws["q2w32"][:], bf16)
        k2w32_sb = wtile("k2w32", [16, 32], dws["k2w32"][:], bf16)
        q2k2b_sb = wtile("q2k2b", [128, 2], dws["q2k2b_rep"][:], f32)
        ax_wT_sb = wtile("axwT", [16, 256], dws["ax_wT"][:], bf16)
        axbh_sb = [wtile(f"axbh{ct}", [128, 1], dws["axb_half"][ct], f32) for ct in range(CT)]
        pw1_sb = [wtile(f"pw1{ct}", [128, 16], dws["pw1_wT"][ct], bf16) for ct in range(CT)]
        pw1b_sb = wtile("pw1b", [128, 1], dws["pw1b_rep"][:], f32)
        mask16_sb = wtile("mask16", [128, 16], dws["mask16"][:], f32)
        pw2_sb = wtile("pw2", [16, 256], dws["pw2_wT"][:], f32)
        pw2bh_sb = [wtile(f"pw2bh{ct}", [128, 1], dws["pw2b_half"][ct], f32) for ct in range(CT)]
        fc1_sb = [wtile(f"fc1{ct}", [128, 16], dws["fc1_wT"][ct], f32) for ct in range(CT)]
        fc1b_sb = wtile("fc1b", [16, 1], dws["fc1b"][:], f32)
        fc2_sb = wtile("fc2", [16, 256], dws["fc2_wT"][:], f32)
        fc2bh_sb = [wtile(f"fc2bh{ct}", [128, 1], dws["fc2b_half"][ct], f32) for ct in range(CT)]
        ident_sb = wtile("ident", [128, 128], dws["ident"][:], bf16)

        for i in range(n_imgs):
            # ================= load x =================
            x = [xp.tile([128, HW], bf16, tag=f"x{ct}", name=f"x{ct}") for ct in range(CT)]
            for ct in range(CT):
                nc.sync.dma_start(out=x[ct][:], in_=dx[i, 128 * ct:128 * (ct + 1), :])

            # ================= global SE gate =================
            tg = []
            if do_se:
                gsum = [tiny.tile([128, 1], f32, tag="gsum", name="gsum") for _ in range(CT)]
                for ct in range(CT):
                    nc.vector.reduce_sum(out=gsum[ct][:], in_=x[ct][:], axis=AX)
                fc1ps = psmall[0:16, 0:1]
                for ct in range(CT):
                    nc.tensor.matmul(fc1ps, fc1_sb[ct][:], gsum[ct][:],
                                     start=(ct == 0), stop=(ct == 1))
                r1 = tiny.tile([16, 1], f32, tag="r1", name="r1")
                nc.scalar.activation(out=r1[:], in_=fc1ps, func=AF.Relu,
                                     bias=fc1b_sb[:], scale=1.0)
                for ct in range(CT):
                    fc2ps = psmall[:, 1 + ct:2 + ct]
                    nc.tensor.matmul(fc2ps, fc2_sb[:, 128 * ct:128 * (ct + 1)], r1[:])
                    t = tiny.tile([128, 1], f32, tag="tg", name="tg")
                    nc.scalar.activation(out=t[:], in_=fc2ps, func=AF.Tanh,
                                         bias=fc2bh_sb[ct][:], scale=0.5)
                    tg.append(t)
            else:
                for ct in range(CT):
                    t = tiny.tile([128, 1], f32, tag="tg", name="tg")
                    nc.vector.memset(t[:], 0.0)
                    tg.append(t)

            if do_conv:
                # ================= conv branch: dw1 -> gelu -> y1 =================
                y1 = [y1p.tile([128, HW], bf16, tag=f"y1{ct}", name=f"y1{ct}") for ct in range(CT)]
                for ct in range(CT):
                    x3 = x[ct].rearrange("p (h w) -> p h w", w=64)
                    for c in range(8):
                        o = 512 * c
                        ps = ps_mm.tile([128, 512], f32, tag="mm", name="mm")
                        ps3 = ps.rearrange("p (h w) -> p h w", w=64)
                        # flat shifted taps; h-row wrap fixed by corrections below
                        nc.tensor.matmul(ps[:], dwd_sb[0][1][ct][:], x[ct][:, o:o + 512],
                                         start=True, stop=False)
                        lo = 1 if c == 0 else 0
                        nc.tensor.matmul(ps[:, lo:512], dwd_sb[0][0][ct][:],
                                         x[ct][:, o + lo - 1:o + 511],
                                         start=False, stop=False)
                        hi = 511 if c == 7 else 512
                        nc.tensor.matmul(ps[:, 0:hi], dwd_sb[0][2][ct][:],
                                         x[ct][:, o + 1:o + 1 + hi],
                                         start=False, stop=True)
                        # subtract wrapped left tap at w=0 (h>0), right tap at w=63
                        lh = 1 if c == 0 else 0
                        nc.vector.scalar_tensor_tensor(
                            out=ps3[:, lh:8, 0], in0=x3[:, 8 * c + lh - 1:8 * c + 7, 63],
                            scalar=dwn_sb[0][ct][:], in1=ps3[:, lh:8, 0],
                            op0=OP.mult, op1=OP.add)
                        rh = 7 if c == 7 else 8
                        nc.vector.scalar_tensor_tensor(
                            out=ps3[:, 0:rh, 63], in0=x3[:, 8 * c + 1:8 * c + 1 + rh, 0],
                            scalar=dwn_sb[1][ct][:], in1=ps3[:, 0:rh, 63],
                            op0=OP.mult, op1=OP.add)
                        nc.scalar.activation(out=y1[ct][:, o:o + 512], in_=ps[:],
                                             func=AF.Gelu, bias=dwb_sb[0][ct][:], scale=1.0)

                # ========== dw2 -> gelu -> y2 chunks -> pw1 (packed) ==========
                # pw1 outputs pack 4 chunks per [128,512] PSUM tile at
                # 32-aligned partition offsets (matmul PSUM writes must be
                # 32-aligned); two passes share one bank via tag rotation
                lacc = tiny.tile([128, 2], f32, tag="lacc", name="lacc")
                pwps = ps_pw.tile([128, 512], f32, tag="pw", name="pw")
                for c in range(8):
                    o = 512 * c
                    y2c = []
                    for ct in range(CT):
                        ps = ps_mm.tile([128, 512], f32, tag="mm", name="mm")
                        nc.tensor.matmul(ps[:], dwd_sb[1][1][ct][:], y1[ct][:, o:o + 512],
                                         start=True, stop=False)
                        if c == 0:
                            nc.tensor.matmul(ps[:, 64:512], dwd_sb[1][0][ct][:],
                                             y1[ct][:, 0:448], start=False, stop=False)
                        else:
                            nc.tensor.matmul(ps[:], dwd_sb[1][0][ct][:],
                                             y1[ct][:, o - 64:o + 448],
                                             start=False, stop=False)
                        if c == 7:
                            nc.tensor.matmul(ps[:, 0:448], dwd_sb[1][2][ct][:],
                                             y1[ct][:, o + 64:o + 512],
                                             start=False, stop=True)
                        else:
                            nc.tensor.matmul(ps[:], dwd_sb[1][2][ct][:],
                                             y1[ct][:, o + 64:o + 576],
                                             start=False, stop=True)
                        yc = scr.tile([128, 512], bf16, tag=f"y2c{ct}", name=f"y2c{ct}")
                        nc.scalar.activation(out=yc[:], in_=ps[:], func=AF.Gelu,
                                             bias=dwb_sb[1][ct][:], scale=1.0)
                        y2c.append(yc)
                    # pw1 for chunk c -> partitions 32*(c%4) of the packed tile
                    po = 32 * (c % 4)
                    for ct in range(CT):
                        nc.tensor.matmul(pwps[po:po + 16, :], pw1_sb[ct][:], y2c[ct][:],
                                         start=(ct == 0), stop=(ct == 1),
                                         tile_position=(0, po))
                    if c == 3 or c == 7:
                        g3 = scr.tile([128, 512], bf16, tag="g3", name="g3")
                        nc.scalar.activation(out=g3[:], in_=pwps[:], func=AF.Gelu,
                                             bias=pw1b_sb[:], scale=1.0,
                                             accum_out=lacc[:, c // 4:c // 4 + 1])
                        if c == 3:
                            pwps = ps_pw.tile([128, 512], f32, tag="pw", name="pw")

                # local gate: partition-reduce acc via mask matmul, then pw2
                lsps = psmall[0:16, 3:4]
                nc.tensor.matmul(lsps, mask16_sb[:], lacc[:, 0:1],
                                 start=True, stop=False)
                nc.tensor.matmul(lsps, mask16_sb[:], lacc[:, 1:2],
                                 start=False, stop=True)
                lsum = tiny.tile([16, 1], f32, tag="lsum", name="lsum")
                nc.vector.tensor_copy(out=lsum[:], in_=lsps)
                tl = []
                for ct in range(CT):
                    ps = psmall[:, 4 + ct:5 + ct]
                    nc.tensor.matmul(ps, pw2_sb[:, 128 * ct:128 * (ct + 1)], lsum[:])
                    t = tiny.tile([128, 1], f32, tag="tl", name="tl")
                    nc.scalar.activation(out=t[:], in_=ps, func=AF.Tanh,
                                         bias=pw2bh_sb[ct][:], scale=0.5)
                    tl.append(t)
            else:
                tl = []
                for ct in range(CT):
                    t = tiny.tile([128, 1], f32, tag="tl", name="tl")
                    nc.vector.memset(t[:], 0.0)
                    tl.append(t)

            if do_att >= 1:
                # ================= row attention =================
                # q/k projections, quad-replicated partition blocks
                q_sb = qkp.tile([128, 1024], bf16, tag="q", name="q")
                k_sb = qkp.tile([128, 1024], bf16, tag="k", name="k")
                for g in range(2):
                    for which, rep, dst, bcol in ((0, qrep_sb, q_sb, 0),
                                                  (1, krep_sb, k_sb, 1)):
                        ps = ps_mm.tile([128, 512], f32, tag="mm", name="mm")
                        for r in range(4):
                            c = 4 * g + r
                            for ct in range(CT):
                                nc.tensor.matmul(
                                    ps[32 * r:32 * r + 32, :],
                                    rep[ct][:, 32 * r:32 * r + 32],
                                    x[ct][:, 512 * c:512 * c + 512],
                                    start=(ct == 0), stop=(ct == 1),
                                    tile_position=(0, 32 * r))
                        nc.vector.tensor_scalar(
                            out=dst[:, 512 * g:512 * g + 512], in0=ps[:],
                            scalar1=qkb_sb[:, bcol:bcol + 1], scalar2=None,
                            op0=OP.add)

                # v-direct: [w, d] layout, pairs packed even/odd partition halves
                vt_sb = att.tile([128, 544], bf16, tag="vt", name="vt")
                vt3 = vt_sb.rearrange("p (j c) -> p j c", c=17)
                nc.vector.memset(vt3[:, :, 16], 1.0)
                for g4 in range(8):
                    vps = psmall[:, 64 + 64 * (g4 % 2):128 + 64 * (g4 % 2)]
                    for p in range(4):
                        j = 4 * g4 + p
                        for h in (2 * j, 2 * j + 1):
                            for ct in range(CT):
                                nc.tensor.matmul(
                                    vps[64 * (h % 2):64 * (h % 2) + 64, 16 * p:16 * p + 16],
                                    x[ct][:, 64 * h:64 * h + 64],
                                    qkv_sb[ct][:, 32:48],
                                    start=(ct == 0), stop=(ct == 1),
                                    tile_position=(0, 64 * (h % 2)))
                    vsrc = vps.rearrange("p (j c) -> p j c", c=16)
                    nc.vector.tensor_copy(out=vt3[:, 4 * g4:4 * g4 + 4, 0:16], in_=vsrc)

                def attention_block(qt, kt, vtt, OC_dst):
                    """S^T matmuls -> exp -> attn@v + denom -> normalize.
                    One S-tile per 64-px chunk so all matmuls in a PSUM bank
                    share one row group (HW: different row groups must not
                    write the same bank)."""
                    vt3l = vtt.rearrange("p (j c) -> p j c", c=17)
                    for t in range(4):
                        expSs = []
                        for half in range(4):
                            cch = 2 * t + half // 2
                            r, g = cch % 4, cch // 4
                            Sps = ps_S.tile([128, 256], f32, tag="S", name="S")
                            for u in range(2):
                                j = 4 * cch + 2 * (half % 2) + u
                                h0 = 2 * j
                                sl = slice(32 * r, 32 * r + 16)
                                fo = 512 * g + 64 * (h0 % 8)
                                # merged even/odd pair matmul: diag blocks are
                                # S^T(h0) rows 0:64 / S^T(h0+1) rows 64:128
                                nc.tensor.matmul(
                                    Sps[:, 128 * u:128 * u + 128],
                                    kt[sl, fo:fo + 128], qt[sl, fo:fo + 128],
                                    tile_position=(32 * r, 0))
                            expS = attS.tile([128, 256], bf16, tag="expS", name="expS")
                            nc.scalar.activation(out=expS[:], in_=Sps[:], func=AF.Exp,
                                                 scale=0.25)
                            expSs.append(expS)
                        Ops = ps_O.tile([128, 136], f32, tag="O", name="O")
                        for s in range(8):
                            j = 8 * t + s
                            expS = expSs[s // 2]
                            u = j % 2
                            for dh in range(2):
                                sl = slice(64 * dh, 64 * dh + 64)
                                E = expS[sl, 128 * u + 64 * dh:128 * u + 64 * dh + 64]
                                nc.tensor.matmul(
                                    Ops[sl, 17 * s:17 * s + 17], E,
                                    vt3l[sl, j, :],
                                    tile_position=(64 * dh, 64 * dh))
                        O3 = Ops.rearrange("p (s c) -> p s c", c=17)
                        rD = tiny.tile([128, 8], f32, tag="rD", name="rD")
                        nc.vector.reciprocal(out=rD[:], in_=O3[:, :, 16])
                        import concourse.bass as bass_mod
                        rDb = bass_mod.AP(tensor=rD.tensor, offset=rD.offset,
                                          ap=[rD.ap[0], [1, 8], [0, 16]])
                        dst3 = OC_dst[:, 128 * t:128 * t + 128].rearrange(
                            "p (s c) -> p s c", c=16)
                        nc.vector.tensor_tensor(out=dst3[:], in0=O3[:, :, 0:16],
                                                in1=rDb, op=OP.mult)

                OR_sb = att.tile([128, 512], bf16, tag="OR", name="OR")
                attention_block(q_sb, k_sb, vt_sb, OR_sb)

                # transposeA + scatter -> XC [16, w*64+h] (w-major)
                XC = xcp.tile([16, HW], bf16, tag="XC", name="XC")
                XC3 = XC.rearrange("d (w h) -> d w h", h=64)
                for j in range(32):
                    trp = psmall_bf[0:16, 384 + 128 * (j % 4):512 + 128 * (j % 4)]
                    nc.tensor.transpose(trp, OR_sb[:, 16 * j:16 * j + 16],
                                        ident_sb[:])
                    tsrc = trp.rearrange("d (t w) -> d w t", t=2)
                    nc.vector.tensor_copy(out=XC3[:, :, 2 * j:2 * j + 2], in_=tsrc)

            if do_att >= 2:
                # ================= col attention =================
                q2_sb = qkp.tile([128, 1024], bf16, tag="q2", name="q2")
                k2_sb = qkp.tile([128, 1024], bf16, tag="k2", name="k2")
                for g in range(2):
                    for w32, dst, bcol in ((q2w32_sb, q2_sb, 0),
                                           (k2w32_sb, k2_sb, 1)):
                        ps = ps_mm.tile([128, 512], f32, tag="mm", name="mm")
                        for r in range(4):
                            c = 4 * g + r
                            nc.tensor.matmul(ps[32 * r:32 * r + 32, :],
                                             w32[:],
                                             XC[:, 512 * c:512 * c + 512],
                                             tile_position=(0, 32 * r))
                        nc.vector.tensor_scalar(
                            out=dst[:, 512 * g:512 * g + 512], in0=ps[:],
                            scalar1=q2k2b_sb[:, bcol:bcol + 1], scalar2=None,
                            op0=OP.add)

                vt2_sb = att.tile([128, 544], bf16, tag="vt2", name="vt2")
                vt23 = vt2_sb.rearrange("p (j c) -> p j c", c=17)
                nc.vector.memset(vt23[:, :, 16], 1.0)
                for g4 in range(8):
                    vps = psmall[:, 64 + 64 * (g4 % 2):128 + 64 * (g4 % 2)]
                    for p in range(4):
                        j = 4 * g4 + p
                        for w in (2 * j, 2 * j + 1):
                            nc.tensor.matmul(
                                vps[64 * (w % 2):64 * (w % 2) + 64, 16 * p:16 * p + 16],
                                XC[:, 64 * w:64 * w + 64], w2_sb[:, 32:48],
                                tile_position=(0, 64 * (w % 2)))
                    vsrc = vps.rearrange("p (j c) -> p j c", c=16)
                    nc.vector.tensor_copy(out=vt23[:, 4 * g4:4 * g4 + 4, 0:16], in_=vsrc)

                OC_sb = att.tile([128, 512], bf16, tag="OC", name="OC")
                attention_block(q2_sb, k2_sb, vt2_sb, OC_sb)

                # transposeB + contiguous scatter -> XC2 [16, pixels]
                XC2 = xcp.tile([16, HW], bf16, tag="XC2", name="XC2")
                for j in range(32):
                    trp = psmall_bf[0:16, 384 + 128 * (j % 4):512 + 128 * (j % 4)]
                    nc.tensor.transpose(trp, OC_sb[:, 16 * j:16 * j + 16],
                                        ident_sb[:])
                    nc.vector.tensor_copy(out=XC2[:, 128 * j:128 * j + 128],
                                          in_=trp)

                # ax projection + tanh + accumulated mean
                ta_cols = [tiny.tile([128, 8], f32, tag="ta_cols", name="ta_cols") for _ in range(CT)]
                for ct in range(CT):
                    for c in range(8):
                        ps = ps_mm.tile([128, 512], f32, tag="mm", name="mm")
                        nc.tensor.matmul(ps[:], ax_wT_sb[:, 128 * ct:128 * (ct + 1)],
                                         XC2[:, 512 * c:512 * c + 512])
                        axs = scr.tile([128, 512], bf16, tag="axs", name="axs")
                        nc.scalar.activation(out=axs[:], in_=ps[:], func=AF.Tanh,
                                             bias=axbh_sb[ct][:], scale=0.5,
                                             accum_out=ta_cols[ct][:, c:c + 1])

            if do_att < 2:
                ta_cols = [tiny.tile([128, 8], f32, tag="ta_cols", name="ta_cols") for _ in range(CT)]
                for ct in range(CT):
                    nc.vector.memset(ta_cols[ct][:], 0.0)
                if do_att == 1:
                    nc.sync.dma_start(out=dout[i, 0:16, :], in_=XC[:])

            # ================= fusion + final =================
            for ct in range(CT):
                ta = tiny.tile([128, 1], f32, tag="ta", name="ta")
                nc.vector.reduce_sum(out=ta[:], in_=ta_cols[ct][:], axis=AX)
                f0 = tiny.tile([128, 1], f32, tag="f0", name="f0")
                nc.vector.tensor_scalar(out=f0[:], in0=tg[ct][:], scalar1=s_g,
                                        scalar2=K0, op0=OP.mult, op1=OP.add)
                f1 = tiny.tile([128, 1], f32, tag="f1", name="f1")
                nc.vector.scalar_tensor_tensor(out=f1[:], in0=tl[ct][:],
                                               scalar=s_l, in1=f0[:],
                                               op0=OP.mult, op1=OP.add)
                fin = tiny.tile([128, 1], f32, tag="fin", name="fin")
                nc.vector.scalar_tensor_tensor(out=fin[:], in0=ta[:],
                                               scalar=s_ax, in1=f1[:],
                                               op0=OP.mult, op1=OP.add)
                nc.vector.tensor_scalar(out=x[ct][:], in0=x[ct][:],
                                        scalar1=fin[:], scalar2=None,
                                        op0=OP.mult)
                nc.sync.dma_start(out=dout[i, 128 * ct:128 * (ct + 1), :],
                                  in_=x[ct][:])

    nc.compile()
    return nc


# ----------------------------------------------------------------------------
# Entry point
# ----------------------------------------------------------------------------
def kernel(**inputs):
    from concourse.bass_utils import run_bass_kernel_spmd

    p = host_prep(inputs)
    key = "nc"
    if key not in _cache:
        _cache[key] = build_nc(p)
    nc = _cache[key]

    x = np.asarray(inputs["x"], np.float32).reshape(B, C, HW).astype(BF)
    wmap = {nm: p[nm] for nm in BF16_W + F32_W}
    in_maps = [{"x": x[IMGS * c:IMGS * (c + 1)], **wmap} for c in range(NCORES)]
    res = run_bass_kernel_spmd(nc, in_maps, list(range(NCORES)))
    _cache["last_results"] = res
    out = np.concatenate([res.results[c]["out"] for c in range(NCORES)], axis=0)
    return out.reshape(B, C, H, W).astype(np.float32)
